# revision 6
# baseline (speedup 1.0000x reference)
"""Trainium2 Bass kernel for nn_Encoder_Decoder_fc (encoder LSTM -> decoder LSTMCell + Linear).

Strategy: data-parallel over batch (B=256 -> 32 per core on 8 cores), weights replicated.

Gates are computed in a transposed ("GT") layout: gate rows live on PSUM
partitions and batch in the free dim, one PSUM tile per gate in fold order
[g | f | i | o] (torch row bases g=1024, f=512, i=0, o=1536). Gate chunk
m = 4*c + jj covers rows base_c + 128*jj, held as tile_c[:, 32*jj + b].
Each 32-wide region accumulates 5 matmuls: one K=2 input+bias term
(lhsT = [Wih_m; bias_m], rhs = [x_t; 1]) and four K=128 recurrent terms
(lhsT = Whh^T chunk [h-dim, gate-dim], rhs = h^T chunk, N=32). Because
gate rows live on partitions, h = sig(o) * tanh(c) lands directly in the
h^T layout the next step's matmuls stream as rhs - no PE transposes.

The serial recurrence chain per step is: matmul burst -> per-gate
activations (tanh_g first, during the burst; sig_f / sig_i staggered so the
DVE c-update ops each fire on their producer's ack) -> c = sig_f*c +
sig_i*tanh_g -> tanh(c) -> h. Gate order, one-PSUM-tile-per-gate (avoids
false tile-granular WAR serialization), prefetched input matmuls, and bf16
activation outputs (DVE 2x mode) are all chain-latency optimizations.
PSUM start=True is issued only on the first matmul per bank: start marks
the whole bank pending-zero, so a second start would drop earlier regions.

The output Linear runs as 4 tiny matmuls (N=32) per decoder step into a
PSUM window flushed every 16 steps via DVE copy (adding lin_b) + DMA.
Step t's y matmuls are emitted after step t+1's recurrent burst so the
in-order PE queue runs them in the idle tail instead of delaying the burst.
"""

import sys

sys.path.insert(0, "/opt/trn_rl_repo")

from contextlib import ExitStack

import ml_dtypes
import numpy as np

import concourse.bass as bass
import concourse.mybir as mybir
import concourse.tile as tile
from concourse import bacc
from concourse.bass_utils import run_bass_kernel_spmd

P = 128
H = 512
B = 256
T = 512
N_CORES = 8
BL = B // N_CORES  # 32 batch per core
KC = H // P  # 4 h-dim chunks
MC = 16  # gate chunks of 128 rows
GW = MC * BL  # 512: G free width
WIN = 16  # ys window size (steps)

F32 = mybir.dt.float32
BF16 = mybir.dt.bfloat16
AF = mybir.ActivationFunctionType

# fold order along m: g, f, i, o ; torch row offsets: i=0, f=512, g=1024, o=1536
# g first so tanh(g) runs during the matmul burst; f next so the c update can
# start early; o last (only needed late, for h = sig(o)*tanh(c)).
_CBASE = (2 * H, 1 * H, 0 * H, 3 * H)  # g, f, i, o


def _perm_fold() -> np.ndarray:
    """perm[128*m + p] = torch row index for folded gate chunk m, row p."""
    idx = np.empty(4 * H, dtype=np.int64)
    for m in range(MC):
        c, jj = divmod(m, KC)
        idx[128 * m : 128 * (m + 1)] = _CBASE[c] + 128 * jj + np.arange(P)
    return idx


def _step(
    nc,
    pools,
    consts,
    t_abs,
    h_prev,
    sWT,
    sUB,
    c_tile,
    first_step,
    skip_rec,
    after_inputs=None,
):
    """One LSTM step in GT layout. Returns the new h^T tile [128, 128] bf16."""
    gpool, g3pool, apool, spool, hpool = (
        pools["g"],
        pools["g3"],
        pools["a"],
        pools["s"],
        pools["h"],
    )
    sXT = consts["XT"]

    xt2 = sXT[:, t_abs * BL : (t_abs + 1) * BL]  # [2, 32]: row0 = x_t, row1 = 1
    # one PSUM tile + one SBUF activation tile per gate [g, f, i, o]: tile-
    # granular dependency tracking would otherwise serialize the next gate's
    # matmuls behind this gate's activation read (false WAR on a shared tile)
    Gs = [
        (g3pool if j == 3 else gpool).tile(
            [P, KC * BL], F32, tag=f"G{j}", name=f"G{j}"
        )
        for j in range(4)
    ]
    # input+bias matmuls first: no h dependence, they run during the previous
    # step's tail while the PE is otherwise idle
    for m in range(MC):
        # start=True only on the first matmul touching each PSUM bank: start
        # marks the whole bank pending-zero (lazily cleared on write), so a
        # second start in the same bank would discard already-written regions
        nc.tensor.matmul(
            Gs[m // 4][:, BL * (m % 4) : BL * (m % 4 + 1)],
            sUB[:, P * m : P * (m + 1)],
            xt2,
            start=(m % 4 == 0),
            stop=skip_rec,
            skip_group_check=True,
        )
    # recurrent matmuls m-outer so gate regions complete progressively; each
    # gate's activation is emitted as soon as its region's matmuls are queued
    As = [apool.tile([P, P], BF16, tag=f"A{j}", name=f"A{j}") for j in range(4)]
    for m in range(MC):
        if not skip_rec:
            reg = Gs[m // 4][:, BL * (m % 4) : BL * (m % 4 + 1)]
            for k in range(KC):
                nc.tensor.matmul(
                    reg,
                    sWT[k][:, P * m : P * (m + 1)],
                    h_prev[:, BL * k : BL * (k + 1)],
                    start=False,
                    stop=(k == KC - 1),
                    skip_group_check=True,
                )
    if after_inputs is not None:
        # previous decoder step's y matmuls: emitted after this step's burst
        # so the in-order PE queue runs them during the tail, where the PE is
        # idle, instead of delaying the burst's first matmuls
        after_inputs()
    Ag, Af, Ai, Ao = As
    tmp = None if first_step else spool.tile([P, P], BF16, tag="tmp")
    for m in range(MC):
        if m % 4 != 3:
            continue
        j = m // 4
        func = AF.Tanh if j == 0 else AF.Sigmoid
        nc.scalar.activation(As[j], Gs[j], func)
        # chain DVE ops emitted right behind their producing activations
        if j == 1 and not first_step:
            nc.vector.tensor_mul(c_tile, Af, c_tile)  # c *= sig(f)
        elif j == 2:
            if first_step:
                # c_prev = 0: c = sig(i) * tanh(g)
                nc.vector.tensor_mul(c_tile, Ai, Ag)
            else:
                nc.vector.tensor_mul(tmp, Ai, Ag)  # all-bf16: DVE 2x mode
                nc.vector.tensor_add(c_tile, c_tile, tmp)

    tct = spool.tile([P, P], BF16, tag="tct")
    nc.scalar.activation(tct, c_tile, AF.Tanh)
    h_new = hpool.tile([P, P], BF16, tag="h")
    nc.vector.tensor_mul(h_new, Ao, tct)  # all-bf16: DVE 2x mode
    return h_new


def build_nc(t_enc=T, t_dec=T, mm_dtype="bf16"):
    assert mm_dtype == "bf16"
    nc = bacc.Bacc()

    tmax = max(t_enc, t_dec)
    dXT = nc.declare_dram_parameter("XT", [2, tmax * BL], BF16, isOutput=False)
    dWE = nc.declare_dram_parameter("WE", [KC, P, 4 * H], BF16, isOutput=False)
    dWD = nc.declare_dram_parameter("WD", [KC, P, 4 * H], BF16, isOutput=False)
    dUE = nc.declare_dram_parameter("UE", [2, 4 * H], BF16, isOutput=False)
    dUD = nc.declare_dram_parameter("UD", [2, 4 * H], BF16, isOutput=False)
    dLW = nc.declare_dram_parameter("LW", [P, KC], BF16, isOutput=False)
    dLB = nc.declare_dram_parameter("LB", [1, 1], F32, isOutput=False)
    dY = nc.declare_dram_parameter("Y", [1, t_dec * BL], F32, isOutput=True)

    with ExitStack() as ctx:
        tc = ctx.enter_context(tile.TileContext(nc))
        const = ctx.enter_context(tc.tile_pool(name="const", bufs=1))
        gpool = ctx.enter_context(tc.tile_pool(name="g", bufs=2, space="PSUM"))
        g3pool = ctx.enter_context(tc.tile_pool(name="g3", bufs=1, space="PSUM"))
        ypool = ctx.enter_context(tc.tile_pool(name="yps", bufs=1, space="PSUM"))
        apool = ctx.enter_context(tc.tile_pool(name="act", bufs=4))
        spool = ctx.enter_context(tc.tile_pool(name="small", bufs=4))
        hpool = ctx.enter_context(tc.tile_pool(name="h", bufs=4))
        ysb_pool = ctx.enter_context(tc.tile_pool(name="ysb", bufs=2))

        # persistent SBUF tensors
        sXT = const.tile([2, tmax * BL], BF16, tag="sXT")
        sWE = [
            const.tile([P, 4 * H], BF16, tag=f"sWE{k}", name=f"sWE{k}")
            for k in range(KC)
        ]
        sWD = [
            const.tile([P, 4 * H], BF16, tag=f"sWD{k}", name=f"sWD{k}")
            for k in range(KC)
        ]
        sUE = const.tile([2, 4 * H], BF16, tag="sUE")
        sUD = const.tile([2, 4 * H], BF16, tag="sUD")
        sLW = const.tile([P, KC], BF16, tag="sLW")
        sLB = const.tile([1, 1], F32, tag="sLB")
        c_tile = const.tile([P, P], BF16, tag="c")

        nc.sync.dma_start(sXT[:, :], dXT[:, :])
        for k in range(KC):
            nc.sync.dma_start(sWE[k][:, :], dWE[k])
            nc.sync.dma_start(sWD[k][:, :], dWD[k])
        nc.sync.dma_start(sUE[:, :], dUE[:, :])
        nc.sync.dma_start(sUD[:, :], dUD[:, :])
        nc.sync.dma_start(sLW[:, :], dLW[:, :])
        nc.sync.dma_start(sLB[:, :], dLB[:, :])

        pools = {
            "g": gpool,
            "g3": g3pool,
            "a": apool,
            "s": spool,
            "h": hpool,
        }
        consts = {"XT": sXT}

        # ---------------- encoder ----------------
        h_prev = None
        for t in range(t_enc):
            h_prev = _step(
                nc,
                pools,
                consts,
                t,
                h_prev,
                sWE,
                sUE,
                c_tile,
                first_step=(t == 0),
                skip_rec=(t == 0),
            )

        # ---------------- decoder ----------------
        yps = None

        def _emit_y(t, h_t):
            """y_t = lin_W @ h_t into the PSUM window."""
            nonlocal yps
            s = t % WIN
            if s == 0:
                yps = ypool.tile([1, WIN * BL], F32, tag="yps")
            yreg = yps[0:1, s * BL : (s + 1) * BL]
            for k in range(KC):
                nc.tensor.matmul(
                    yreg,
                    sLW[:, k : k + 1],
                    h_t[:, BL * k : BL * (k + 1)],
                    start=(k == 0),
                    stop=(k == KC - 1),
                    skip_group_check=True,
                )

        def _flush_y(t):
            """Flush the window holding y_t. Runs on ACT (Identity + lin_b
            bias), emitted after tanh(c) so it never head-of-line-blocks the
            chain's DVE ops or the next step's gate activations."""
            w = t // WIN
            n = t % WIN + 1
            ysb = ysb_pool.tile([1, WIN * BL], F32, tag="ysb")
            nc.scalar.activation(
                ysb[0:1, 0 : n * BL],
                yps[0:1, 0 : n * BL],
                AF.Identity,
                bias=sLB[0:1, 0:1],
            )
            nc.sync.dma_start(
                dY[0:1, w * WIN * BL : w * WIN * BL + n * BL],
                ysb[0:1, 0 : n * BL],
            )

        for t in range(t_dec):
            h_last = h_prev
            h_prev = _step(
                nc,
                pools,
                consts,
                t,
                h_prev,
                sWD,
                sUD,
                c_tile,
                first_step=(t == 0),
                skip_rec=False,
                # y matmuls for step t-1 wait on h(t-1); queue them behind
                # this step's prefetched input matmuls, not ahead of them
                after_inputs=(
                    (lambda tt=t - 1, hh=h_last: _emit_y(tt, hh)) if t > 0 else None
                ),
            )
            if t > 0 and (t - 1) % WIN == WIN - 1:
                _flush_y(t - 1)
        _emit_y(t_dec - 1, h_prev)
        _flush_y(t_dec - 1)

    if not nc.is_finalized():
        nc.finalize()
    return nc


def prep_core_inputs(x_core, weights, mm_dtype="bf16"):
    """Host-side layout prep for one core. x_core: [BL, T, 1] fp32."""
    perm = _perm_fold()
    out = {}
    xt = np.zeros((2, T * BL), dtype=np.float32)
    xt[0] = x_core[:, :, 0].T.reshape(-1)  # t-major: idx = t*BL + b
    xt[1] = 1.0
    out["XT"] = xt.astype(ml_dtypes.bfloat16)
    for tag, Wih, Whh, bih, bhh in (
        ("E", weights["enc_Wih"], weights["enc_Whh"], weights["enc_bih"], weights["enc_bhh"]),
        ("D", weights["dec_Wih"], weights["dec_Whh"], weights["dec_bih"], weights["dec_bhh"]),
    ):
        Wf = np.asarray(Whh)[perm, :]  # [4H, H] folded gate rows
        # lhsT chunk k: [128 h-dims, 2048 gate rows]
        wt = np.stack([np.ascontiguousarray(Wf[:, P * k : P * (k + 1)].T) for k in range(KC)])
        out["W" + tag] = wt.astype(ml_dtypes.bfloat16)
        u = np.zeros((2, 4 * H), dtype=np.float32)
        u[0] = np.asarray(Wih)[perm, 0]
        u[1] = (np.asarray(bih) + np.asarray(bhh))[perm]
        out["U" + tag] = u.astype(ml_dtypes.bfloat16)
    out["LW"] = np.ascontiguousarray(
        np.asarray(weights["lin_W"])[0].reshape(KC, P).T
    ).astype(ml_dtypes.bfloat16)
    out["LB"] = np.asarray(weights["lin_b"]).reshape(1, 1).astype(np.float32)
    return out


_CACHE = {}
_LAST_RESULTS = None


def kernel(**inputs) -> np.ndarray:
    global _LAST_RESULTS
    key = "full"
    if key not in _CACHE:
        _CACHE[key] = build_nc(T, T)
    nc = _CACHE[key]

    x = np.asarray(inputs["x"], dtype=np.float32)
    in_maps = [
        prep_core_inputs(x[i * BL : (i + 1) * BL], inputs) for i in range(N_CORES)
    ]

    res = run_bass_kernel_spmd(nc, in_maps, core_ids=list(range(N_CORES)))
    _LAST_RESULTS = res
    y = np.empty((B, T, 1), dtype=np.float32)
    for i in range(N_CORES):
        yi = np.asarray(res.results[i]["Y"], dtype=np.float32).reshape(T, BL)
        y[i * BL : (i + 1) * BL, :, 0] = yi.T
    return y


# revision 7
# speedup vs baseline: 1.0029x; 1.0029x over previous
"""Trainium2 Bass kernel for nn_Encoder_Decoder_fc (encoder LSTM -> decoder LSTMCell + Linear).

Strategy: data-parallel over batch (B=256 -> 32 per core on 8 cores), weights replicated.

Gates are computed in a transposed ("GT") layout: gate rows live on PSUM
partitions and batch in the free dim, one PSUM tile per gate in fold order
[g | f | i | o] (torch row bases g=1024, f=512, i=0, o=1536). Gate chunk
m = 4*c + jj covers rows base_c + 128*jj, held as tile_c[:, 32*jj + b].
Each 32-wide region accumulates 5 matmuls: one K=2 input+bias term
(lhsT = [Wih_m; bias_m], rhs = [x_t; 1]) and four K=128 recurrent terms
(lhsT = Whh^T chunk [h-dim, gate-dim], rhs = h^T chunk, N=32). Because
gate rows live on partitions, h = sig(o) * tanh(c) lands directly in the
h^T layout the next step's matmuls stream as rhs - no PE transposes.

The serial recurrence chain per step is: matmul burst -> per-gate
activations (tanh_g first, during the burst; sig_f / sig_i staggered so the
DVE c-update ops each fire on their producer's ack) -> c = sig_f*c +
sig_i*tanh_g -> tanh(c) -> h. Gate order, one-PSUM-tile-per-gate (avoids
false tile-granular WAR serialization), prefetched input matmuls, and bf16
activation outputs (DVE 2x mode) are all chain-latency optimizations.
PSUM start=True is issued only on the first matmul per bank: start marks
the whole bank pending-zero, so a second start would drop earlier regions.

The output Linear runs as 4 tiny matmuls (N=32) per decoder step into a
PSUM window flushed every 16 steps via two half-window ACT Identity+lin_b
ops (each sized to fit the ACT engine's idle gap before tanh(c)) + DMA.
Step t's y matmuls are emitted after step t+1's recurrent burst so the
in-order PE queue runs them in the idle tail instead of delaying the burst.
"""

import sys

sys.path.insert(0, "/opt/trn_rl_repo")

from contextlib import ExitStack

import ml_dtypes
import numpy as np

import concourse.bass as bass
import concourse.mybir as mybir
import concourse.tile as tile
from concourse import bacc
from concourse.bass_utils import run_bass_kernel_spmd

P = 128
H = 512
B = 256
T = 512
N_CORES = 8
BL = B // N_CORES  # 32 batch per core
KC = H // P  # 4 h-dim chunks
MC = 16  # gate chunks of 128 rows
GW = MC * BL  # 512: G free width
WIN = 16  # ys window size (steps)

F32 = mybir.dt.float32
BF16 = mybir.dt.bfloat16
AF = mybir.ActivationFunctionType

# fold order along m: g, f, i, o ; torch row offsets: i=0, f=512, g=1024, o=1536
# g first so tanh(g) runs during the matmul burst; f next so the c update can
# start early; o last (only needed late, for h = sig(o)*tanh(c)).
_CBASE = (2 * H, 1 * H, 0 * H, 3 * H)  # g, f, i, o


def _perm_fold() -> np.ndarray:
    """perm[128*m + p] = torch row index for folded gate chunk m, row p."""
    idx = np.empty(4 * H, dtype=np.int64)
    for m in range(MC):
        c, jj = divmod(m, KC)
        idx[128 * m : 128 * (m + 1)] = _CBASE[c] + 128 * jj + np.arange(P)
    return idx


def _step(
    nc,
    pools,
    consts,
    t_abs,
    h_prev,
    sWT,
    sUB,
    c_tile,
    first_step,
    skip_rec,
    after_inputs=None,
):
    """One LSTM step in GT layout. Returns the new h^T tile [128, 128] bf16."""
    gpool, g3pool, apool, spool, hpool = (
        pools["g"],
        pools["g3"],
        pools["a"],
        pools["s"],
        pools["h"],
    )
    sXT = consts["XT"]

    xt2 = sXT[:, t_abs * BL : (t_abs + 1) * BL]  # [2, 32]: row0 = x_t, row1 = 1
    # one PSUM tile + one SBUF activation tile per gate [g, f, i, o]: tile-
    # granular dependency tracking would otherwise serialize the next gate's
    # matmuls behind this gate's activation read (false WAR on a shared tile)
    Gs = [
        (g3pool if j in (0, 3) else gpool).tile(
            [P, KC * BL], F32, tag=f"G{j}", name=f"G{j}"
        )
        for j in range(4)
    ]
    # input+bias matmuls first: no h dependence, they run during the previous
    # step's tail while the PE is otherwise idle
    for m in range(MC):
        # start=True only on the first matmul touching each PSUM bank: start
        # marks the whole bank pending-zero (lazily cleared on write), so a
        # second start in the same bank would discard already-written regions
        nc.tensor.matmul(
            Gs[m // 4][:, BL * (m % 4) : BL * (m % 4 + 1)],
            sUB[:, P * m : P * (m + 1)],
            xt2,
            start=(m % 4 == 0),
            stop=skip_rec,
            skip_group_check=True,
        )
    # recurrent matmuls m-outer so gate regions complete progressively; each
    # gate's activation is emitted as soon as its region's matmuls are queued
    As = [apool.tile([P, P], BF16, tag=f"A{j}", name=f"A{j}") for j in range(4)]
    for m in range(MC):
        if not skip_rec:
            reg = Gs[m // 4][:, BL * (m % 4) : BL * (m % 4 + 1)]
            for k in range(KC):
                nc.tensor.matmul(
                    reg,
                    sWT[k][:, P * m : P * (m + 1)],
                    h_prev[:, BL * k : BL * (k + 1)],
                    start=False,
                    stop=(k == KC - 1),
                    skip_group_check=True,
                )
    if after_inputs is not None:
        # previous decoder step's y matmuls: emitted after this step's burst
        # so the in-order PE queue runs them during the tail, where the PE is
        # idle, instead of delaying the burst's first matmuls
        after_inputs()
    Ag, Af, Ai, Ao = As
    tmp = None if first_step else spool.tile([P, P], BF16, tag="tmp")
    for m in range(MC):
        if m % 4 != 3:
            continue
        j = m // 4
        func = AF.Tanh if j == 0 else AF.Sigmoid
        nc.scalar.activation(As[j], Gs[j], func)
        # chain DVE ops emitted right behind their producing activations
        if j == 1 and not first_step:
            nc.vector.tensor_mul(c_tile, Af, c_tile)  # c *= sig(f)
        elif j == 2:
            if first_step:
                # c_prev = 0: c = sig(i) * tanh(g)
                nc.vector.tensor_mul(c_tile, Ai, Ag)
            else:
                nc.vector.tensor_mul(tmp, Ai, Ag)  # all-bf16: DVE 2x mode
                nc.vector.tensor_add(c_tile, c_tile, tmp)

    tct = spool.tile([P, P], BF16, tag="tct")
    nc.scalar.activation(tct, c_tile, AF.Tanh)
    h_new = hpool.tile([P, P], BF16, tag="h")
    nc.vector.tensor_mul(h_new, Ao, tct)  # all-bf16: DVE 2x mode
    return h_new


def build_nc(t_enc=T, t_dec=T, mm_dtype="bf16"):
    assert mm_dtype == "bf16"
    nc = bacc.Bacc()

    tmax = max(t_enc, t_dec)
    dXT = nc.declare_dram_parameter("XT", [2, tmax * BL], BF16, isOutput=False)
    dWE = nc.declare_dram_parameter("WE", [KC, P, 4 * H], BF16, isOutput=False)
    dWD = nc.declare_dram_parameter("WD", [KC, P, 4 * H], BF16, isOutput=False)
    dUE = nc.declare_dram_parameter("UE", [2, 4 * H], BF16, isOutput=False)
    dUD = nc.declare_dram_parameter("UD", [2, 4 * H], BF16, isOutput=False)
    dLW = nc.declare_dram_parameter("LW", [P, KC], BF16, isOutput=False)
    dLB = nc.declare_dram_parameter("LB", [1, 1], F32, isOutput=False)
    dY = nc.declare_dram_parameter("Y", [1, t_dec * BL], F32, isOutput=True)

    with ExitStack() as ctx:
        tc = ctx.enter_context(tile.TileContext(nc))
        const = ctx.enter_context(tc.tile_pool(name="const", bufs=1))
        gpool = ctx.enter_context(tc.tile_pool(name="g", bufs=2, space="PSUM"))
        g3pool = ctx.enter_context(tc.tile_pool(name="g3", bufs=1, space="PSUM"))
        ypool = ctx.enter_context(tc.tile_pool(name="yps", bufs=2, space="PSUM"))
        apool = ctx.enter_context(tc.tile_pool(name="act", bufs=4))
        spool = ctx.enter_context(tc.tile_pool(name="small", bufs=4))
        hpool = ctx.enter_context(tc.tile_pool(name="h", bufs=4))
        ysb_pool = ctx.enter_context(tc.tile_pool(name="ysb", bufs=2))

        # persistent SBUF tensors
        sXT = const.tile([2, tmax * BL], BF16, tag="sXT")
        sWE = [
            const.tile([P, 4 * H], BF16, tag=f"sWE{k}", name=f"sWE{k}")
            for k in range(KC)
        ]
        sWD = [
            const.tile([P, 4 * H], BF16, tag=f"sWD{k}", name=f"sWD{k}")
            for k in range(KC)
        ]
        sUE = const.tile([2, 4 * H], BF16, tag="sUE")
        sUD = const.tile([2, 4 * H], BF16, tag="sUD")
        sLW = const.tile([P, KC], BF16, tag="sLW")
        sLB = const.tile([1, 1], F32, tag="sLB")
        c_tile = const.tile([P, P], BF16, tag="c")

        nc.sync.dma_start(sXT[:, :], dXT[:, :])
        for k in range(KC):
            nc.sync.dma_start(sWE[k][:, :], dWE[k])
            nc.sync.dma_start(sWD[k][:, :], dWD[k])
        nc.sync.dma_start(sUE[:, :], dUE[:, :])
        nc.sync.dma_start(sUD[:, :], dUD[:, :])
        nc.sync.dma_start(sLW[:, :], dLW[:, :])
        nc.sync.dma_start(sLB[:, :], dLB[:, :])

        pools = {
            "g": gpool,
            "g3": g3pool,
            "a": apool,
            "s": spool,
            "h": hpool,
        }
        consts = {"XT": sXT}

        # ---------------- encoder ----------------
        h_prev = None
        for t in range(t_enc):
            h_prev = _step(
                nc,
                pools,
                consts,
                t,
                h_prev,
                sWE,
                sUE,
                c_tile,
                first_step=(t == 0),
                skip_rec=(t == 0),
            )

        # ---------------- decoder ----------------
        yps = None

        def _emit_y(t, h_t):
            """y_t = lin_W @ h_t into the PSUM window."""
            nonlocal yps
            s = t % WIN
            if s == 0:
                yps = ypool.tile([1, WIN * BL], F32, tag="yps")
            yreg = yps[0:1, s * BL : (s + 1) * BL]
            for k in range(KC):
                nc.tensor.matmul(
                    yreg,
                    sLW[:, k : k + 1],
                    h_t[:, BL * k : BL * (k + 1)],
                    start=(k == 0),
                    stop=(k == KC - 1),
                    skip_group_check=True,
                )

        def _flush_y(t):
            """Flush the window holding y_t. Runs on ACT (Identity + lin_b
            bias) at deprioritized order so the Tile scheduler never slots it
            ahead of the chain-critical tanh(c) in the ACT FIFO."""
            w = t // WIN
            n = t % WIN + 1
            ysb = ysb_pool.tile([1, WIN * BL], F32, tag="ysb")
            # two half-window chunks: each 398 ns ACT op fits the idle gap
            # between sig_o and tanh(c), so the flush never delays the chain
            for lo in range(0, n, WIN // 2):
                hi = min(n, lo + WIN // 2)
                nc.scalar.activation(
                    ysb[0:1, lo * BL : hi * BL],
                    yps[0:1, lo * BL : hi * BL],
                    AF.Identity,
                    bias=sLB[0:1, 0:1],
                )
            nc.sync.dma_start(
                dY[0:1, w * WIN * BL : w * WIN * BL + n * BL],
                ysb[0:1, 0 : n * BL],
            )

        for t in range(t_dec):
            h_last = h_prev
            h_prev = _step(
                nc,
                pools,
                consts,
                t,
                h_prev,
                sWD,
                sUD,
                c_tile,
                first_step=(t == 0),
                skip_rec=False,
                # y matmuls for step t-1 wait on h(t-1); queue them behind
                # this step's prefetched input matmuls, not ahead of them
                after_inputs=(
                    (lambda tt=t - 1, hh=h_last: _emit_y(tt, hh)) if t > 0 else None
                ),
            )
            if t > 0 and (t - 1) % WIN == WIN - 1:
                _flush_y(t - 1)
        _emit_y(t_dec - 1, h_prev)
        _flush_y(t_dec - 1)

    if not nc.is_finalized():
        nc.finalize()
    return nc


def prep_core_inputs(x_core, weights, mm_dtype="bf16"):
    """Host-side layout prep for one core. x_core: [BL, T, 1] fp32."""
    perm = _perm_fold()
    out = {}
    xt = np.zeros((2, T * BL), dtype=np.float32)
    xt[0] = x_core[:, :, 0].T.reshape(-1)  # t-major: idx = t*BL + b
    xt[1] = 1.0
    out["XT"] = xt.astype(ml_dtypes.bfloat16)
    for tag, Wih, Whh, bih, bhh in (
        ("E", weights["enc_Wih"], weights["enc_Whh"], weights["enc_bih"], weights["enc_bhh"]),
        ("D", weights["dec_Wih"], weights["dec_Whh"], weights["dec_bih"], weights["dec_bhh"]),
    ):
        Wf = np.asarray(Whh)[perm, :]  # [4H, H] folded gate rows
        # lhsT chunk k: [128 h-dims, 2048 gate rows]
        wt = np.stack([np.ascontiguousarray(Wf[:, P * k : P * (k + 1)].T) for k in range(KC)])
        out["W" + tag] = wt.astype(ml_dtypes.bfloat16)
        u = np.zeros((2, 4 * H), dtype=np.float32)
        u[0] = np.asarray(Wih)[perm, 0]
        u[1] = (np.asarray(bih) + np.asarray(bhh))[perm]
        out["U" + tag] = u.astype(ml_dtypes.bfloat16)
    out["LW"] = np.ascontiguousarray(
        np.asarray(weights["lin_W"])[0].reshape(KC, P).T
    ).astype(ml_dtypes.bfloat16)
    out["LB"] = np.asarray(weights["lin_b"]).reshape(1, 1).astype(np.float32)
    return out


_CACHE = {}
_LAST_RESULTS = None


def kernel(**inputs) -> np.ndarray:
    global _LAST_RESULTS
    key = "full"
    if key not in _CACHE:
        _CACHE[key] = build_nc(T, T)
    nc = _CACHE[key]

    x = np.asarray(inputs["x"], dtype=np.float32)
    in_maps = [
        prep_core_inputs(x[i * BL : (i + 1) * BL], inputs) for i in range(N_CORES)
    ]

    res = run_bass_kernel_spmd(nc, in_maps, core_ids=list(range(N_CORES)))
    _LAST_RESULTS = res
    y = np.empty((B, T, 1), dtype=np.float32)
    for i in range(N_CORES):
        yi = np.asarray(res.results[i]["Y"], dtype=np.float32).reshape(T, BL)
        y[i * BL : (i + 1) * BL, :, 0] = yi.T
    return y


# revision 8
# speedup vs baseline: 1.0072x; 1.0043x over previous
"""Trainium2 Bass kernel for nn_Encoder_Decoder_fc (encoder LSTM -> decoder LSTMCell + Linear).

Strategy: data-parallel over batch (B=256 -> 32 per core on 8 cores), weights replicated.

Gates are computed in a transposed ("GT") layout: gate rows live on PSUM
partitions and batch in the free dim, one PSUM tile per gate in fold order
[g | f | i | o] (torch row bases g=1024, f=512, i=0, o=1536). Gate chunk
m = 4*c + jj covers rows base_c + 128*jj, held as tile_c[:, 32*jj + b].
Each 32-wide region accumulates 5 matmuls: one K=2 input+bias term
(lhsT = [Wih_m; bias_m], rhs = [x_t; 1]) and four K=128 recurrent terms
(lhsT = Whh^T chunk [h-dim, gate-dim], rhs = h^T chunk, N=32). Because
gate rows live on partitions, h = sig(o) * tanh(c) lands directly in the
h^T layout the next step's matmuls stream as rhs - no PE transposes.

The serial recurrence chain per step is: matmul burst -> per-gate
activations (tanh_g first, during the burst; sig_f / sig_i staggered so the
DVE c-update ops each fire on their producer's ack) -> c = sig_f*c +
sig_i*tanh_g -> tanh(c) -> h. Gate order, one-PSUM-tile-per-gate (avoids
false tile-granular WAR serialization), prefetched input matmuls, and bf16
activation outputs (DVE 2x mode) are all chain-latency optimizations.
PSUM start=True is issued only on the first matmul per bank: start marks
the whole bank pending-zero, so a second start would drop earlier regions.

The output Linear runs as 4 tiny matmuls (N=32) per decoder step into a
PSUM window flushed every 16 steps via two half-window ACT Identity+lin_b
ops (each sized to fit the ACT engine's idle gap before tanh(c)) + DMA.
Step t's y matmuls are emitted after step t+1's recurrent burst so the
in-order PE queue runs them in the idle tail instead of delaying the burst.
"""

import sys

sys.path.insert(0, "/opt/trn_rl_repo")

from contextlib import ExitStack

import ml_dtypes
import numpy as np

import concourse.bass as bass
import concourse.mybir as mybir
import concourse.tile as tile
from concourse import bacc
from concourse.bass_utils import run_bass_kernel_spmd

P = 128
H = 512
B = 256
T = 512
N_CORES = 8
BL = B // N_CORES  # 32 batch per core
KC = H // P  # 4 h-dim chunks
MC = 16  # gate chunks of 128 rows
GW = MC * BL  # 512: G free width
WIN = 16  # ys window size (steps)

F32 = mybir.dt.float32
BF16 = mybir.dt.bfloat16
AF = mybir.ActivationFunctionType

# fold order along m: g, f, i, o ; torch row offsets: i=0, f=512, g=1024, o=1536
# g first so tanh(g) runs during the matmul burst; f next so the c update can
# start early; o last (only needed late, for h = sig(o)*tanh(c)).
_CBASE = (2 * H, 1 * H, 0 * H, 3 * H)  # g, f, i, o


def _perm_fold() -> np.ndarray:
    """perm[128*m + p] = torch row index for folded gate chunk m, row p."""
    idx = np.empty(4 * H, dtype=np.int64)
    for m in range(MC):
        c, jj = divmod(m, KC)
        idx[128 * m : 128 * (m + 1)] = _CBASE[c] + 128 * jj + np.arange(P)
    return idx


def _step(
    nc,
    pools,
    consts,
    t_abs,
    h_prev,
    sWT,
    sUB,
    c_tile,
    first_step,
    skip_rec,
    after_inputs=None,
):
    """One LSTM step in GT layout. Returns the new h^T tile [128, 128] bf16."""
    gpool, g3pool, apool, spool, hpool = (
        pools["g"],
        pools["g3"],
        pools["a"],
        pools["s"],
        pools["h"],
    )
    sXT = consts["XT"]

    xt2 = sXT[:, t_abs * BL : (t_abs + 1) * BL]  # [2, 32]: row0 = x_t, row1 = 1
    # one PSUM tile + one SBUF activation tile per gate [g, f, i, o]: tile-
    # granular dependency tracking would otherwise serialize the next gate's
    # matmuls behind this gate's activation read (false WAR on a shared tile)
    Gs = [
        (g3pool if j in (0, 3) else gpool).tile(
            [P, KC * BL], F32, tag=f"G{j}", name=f"G{j}"
        )
        for j in range(4)
    ]
    # input+bias matmuls first: no h dependence, they run during the previous
    # step's tail while the PE is otherwise idle
    for m in range(MC):
        # start=True only on the first matmul touching each PSUM bank: start
        # marks the whole bank pending-zero (lazily cleared on write), so a
        # second start in the same bank would discard already-written regions
        nc.tensor.matmul(
            Gs[m // 4][:, BL * (m % 4) : BL * (m % 4 + 1)],
            sUB[:, P * m : P * (m + 1)],
            xt2,
            start=(m % 4 == 0),
            stop=skip_rec,
            skip_group_check=True,
        )
    # recurrent matmuls m-outer so gate regions complete progressively; each
    # gate's activation is emitted as soon as its region's matmuls are queued
    As = [apool.tile([P, P], BF16, tag=f"A{j}", name=f"A{j}") for j in range(4)]
    for m in range(MC):
        if not skip_rec:
            reg = Gs[m // 4][:, BL * (m % 4) : BL * (m % 4 + 1)]
            for k in range(KC):
                nc.tensor.matmul(
                    reg,
                    sWT[k][:, P * m : P * (m + 1)],
                    h_prev[:, BL * k : BL * (k + 1)],
                    start=False,
                    stop=(k == KC - 1),
                    skip_group_check=True,
                )
    if after_inputs is not None:
        # previous decoder step's y matmuls: emitted after this step's burst
        # so the in-order PE queue runs them during the tail, where the PE is
        # idle, instead of delaying the burst's first matmuls
        after_inputs()
    Ag, Af, Ai, Ao = As
    tmp = None if first_step else spool.tile([P, P], BF16, tag="tmp")
    for m in range(MC):
        if m % 4 != 3:
            continue
        j = m // 4
        func = AF.Tanh if j == 0 else AF.Sigmoid
        nc.scalar.activation(As[j], Gs[j], func)
        # chain DVE ops emitted right behind their producing activations
        if j == 1 and not first_step:
            nc.vector.tensor_mul(c_tile, Af, c_tile)  # c *= sig(f)
        elif j == 2:
            if first_step:
                # c_prev = 0: c = sig(i) * tanh(g)
                nc.vector.tensor_mul(c_tile, Ai, Ag)
            else:
                nc.vector.tensor_mul(tmp, Ai, Ag)  # all-bf16: DVE 2x mode
                nc.vector.tensor_add(c_tile, c_tile, tmp)

    tct = spool.tile([P, P], BF16, tag="tct")
    nc.scalar.activation(tct, c_tile, AF.Tanh)
    h_new = hpool.tile([P, P], BF16, tag="h")
    nc.vector.tensor_mul(h_new, Ao, tct)  # all-bf16: DVE 2x mode
    return h_new


def build_nc(t_enc=T, t_dec=T, mm_dtype="bf16"):
    assert mm_dtype == "bf16"
    nc = bacc.Bacc()

    tmax = max(t_enc, t_dec)
    dXT = nc.declare_dram_parameter("XT", [2, tmax * BL], BF16, isOutput=False)
    dWE = nc.declare_dram_parameter("WE", [KC, P, 4 * H], BF16, isOutput=False)
    dWD = nc.declare_dram_parameter("WD", [KC, P, 4 * H], BF16, isOutput=False)
    dUE = nc.declare_dram_parameter("UE", [2, 4 * H], BF16, isOutput=False)
    dUD = nc.declare_dram_parameter("UD", [2, 4 * H], BF16, isOutput=False)
    dLW = nc.declare_dram_parameter("LW", [P, KC], BF16, isOutput=False)
    dLB = nc.declare_dram_parameter("LB", [1, 1], F32, isOutput=False)
    dY = nc.declare_dram_parameter("Y", [1, t_dec * BL], F32, isOutput=True)

    with ExitStack() as ctx:
        tc = ctx.enter_context(tile.TileContext(nc))
        const = ctx.enter_context(tc.tile_pool(name="const", bufs=1))
        gpool = ctx.enter_context(tc.tile_pool(name="g", bufs=2, space="PSUM"))
        g3pool = ctx.enter_context(tc.tile_pool(name="g3", bufs=1, space="PSUM"))
        ypool = ctx.enter_context(tc.tile_pool(name="yps", bufs=2, space="PSUM"))
        apool = ctx.enter_context(tc.tile_pool(name="act", bufs=4))
        spool = ctx.enter_context(tc.tile_pool(name="small", bufs=4))
        hpool = ctx.enter_context(tc.tile_pool(name="h", bufs=4))
        ysb_pool = ctx.enter_context(tc.tile_pool(name="ysb", bufs=2))

        # persistent SBUF tensors
        sXT = const.tile([2, tmax * BL], BF16, tag="sXT")
        sWE = [
            const.tile([P, 4 * H], BF16, tag=f"sWE{k}", name=f"sWE{k}")
            for k in range(KC)
        ]
        sWD = [
            const.tile([P, 4 * H], BF16, tag=f"sWD{k}", name=f"sWD{k}")
            for k in range(KC)
        ]
        sUE = const.tile([2, 4 * H], BF16, tag="sUE")
        sUD = const.tile([2, 4 * H], BF16, tag="sUD")
        sLW = const.tile([P, KC], BF16, tag="sLW")
        sLB = const.tile([1, 1], F32, tag="sLB")
        c_tile = const.tile([P, P], BF16, tag="c")

        # DMA transfers are serialized; issue in first-use order so the first
        # steps aren't gated on data they don't need: x head + encoder weights
        # first, then the x tail, and the decoder weights last
        xhead = min(64 * BL, tmax * BL)
        nc.sync.dma_start(sXT[:, 0:xhead], dXT[:, 0:xhead])
        nc.sync.dma_start(sUE[:, :], dUE[:, :])
        for k in range(KC):
            nc.sync.dma_start(sWE[k][:, :], dWE[k])
        if xhead < tmax * BL:
            nc.sync.dma_start(sXT[:, xhead:], dXT[:, xhead:])
        nc.sync.dma_start(sUD[:, :], dUD[:, :])
        for k in range(KC):
            nc.sync.dma_start(sWD[k][:, :], dWD[k])
        nc.sync.dma_start(sLW[:, :], dLW[:, :])
        nc.sync.dma_start(sLB[:, :], dLB[:, :])

        pools = {
            "g": gpool,
            "g3": g3pool,
            "a": apool,
            "s": spool,
            "h": hpool,
        }
        consts = {"XT": sXT}

        # ---------------- encoder ----------------
        h_prev = None
        for t in range(t_enc):
            h_prev = _step(
                nc,
                pools,
                consts,
                t,
                h_prev,
                sWE,
                sUE,
                c_tile,
                first_step=(t == 0),
                skip_rec=(t == 0),
            )

        # ---------------- decoder ----------------
        yps = None

        def _emit_y(t, h_t):
            """y_t = lin_W @ h_t into the PSUM window."""
            nonlocal yps
            s = t % WIN
            if s == 0:
                yps = ypool.tile([1, WIN * BL], F32, tag="yps")
            yreg = yps[0:1, s * BL : (s + 1) * BL]
            for k in range(KC):
                nc.tensor.matmul(
                    yreg,
                    sLW[:, k : k + 1],
                    h_t[:, BL * k : BL * (k + 1)],
                    start=(k == 0),
                    stop=(k == KC - 1),
                    skip_group_check=True,
                )

        def _flush_y(t):
            """Flush the window holding y_t. Runs on ACT (Identity + lin_b
            bias) at deprioritized order so the Tile scheduler never slots it
            ahead of the chain-critical tanh(c) in the ACT FIFO."""
            w = t // WIN
            n = t % WIN + 1
            ysb = ysb_pool.tile([1, WIN * BL], F32, tag="ysb")
            # two half-window chunks: each 398 ns ACT op fits the idle gap
            # between sig_o and tanh(c), so the flush never delays the chain
            for lo in range(0, n, WIN // 2):
                hi = min(n, lo + WIN // 2)
                nc.scalar.activation(
                    ysb[0:1, lo * BL : hi * BL],
                    yps[0:1, lo * BL : hi * BL],
                    AF.Identity,
                    bias=sLB[0:1, 0:1],
                )
            nc.sync.dma_start(
                dY[0:1, w * WIN * BL : w * WIN * BL + n * BL],
                ysb[0:1, 0 : n * BL],
            )

        for t in range(t_dec):
            h_last = h_prev
            h_prev = _step(
                nc,
                pools,
                consts,
                t,
                h_prev,
                sWD,
                sUD,
                c_tile,
                first_step=(t == 0),
                skip_rec=False,
                # y matmuls for step t-1 wait on h(t-1); queue them behind
                # this step's prefetched input matmuls, not ahead of them
                after_inputs=(
                    (lambda tt=t - 1, hh=h_last: _emit_y(tt, hh)) if t > 0 else None
                ),
            )
            if t > 0 and (t - 1) % WIN == WIN - 1:
                _flush_y(t - 1)
        _emit_y(t_dec - 1, h_prev)
        _flush_y(t_dec - 1)

    if not nc.is_finalized():
        nc.finalize()
    return nc


def prep_core_inputs(x_core, weights, mm_dtype="bf16"):
    """Host-side layout prep for one core. x_core: [BL, T, 1] fp32."""
    perm = _perm_fold()
    out = {}
    xt = np.zeros((2, T * BL), dtype=np.float32)
    xt[0] = x_core[:, :, 0].T.reshape(-1)  # t-major: idx = t*BL + b
    xt[1] = 1.0
    out["XT"] = xt.astype(ml_dtypes.bfloat16)
    for tag, Wih, Whh, bih, bhh in (
        ("E", weights["enc_Wih"], weights["enc_Whh"], weights["enc_bih"], weights["enc_bhh"]),
        ("D", weights["dec_Wih"], weights["dec_Whh"], weights["dec_bih"], weights["dec_bhh"]),
    ):
        Wf = np.asarray(Whh)[perm, :]  # [4H, H] folded gate rows
        # lhsT chunk k: [128 h-dims, 2048 gate rows]
        wt = np.stack([np.ascontiguousarray(Wf[:, P * k : P * (k + 1)].T) for k in range(KC)])
        out["W" + tag] = wt.astype(ml_dtypes.bfloat16)
        u = np.zeros((2, 4 * H), dtype=np.float32)
        u[0] = np.asarray(Wih)[perm, 0]
        u[1] = (np.asarray(bih) + np.asarray(bhh))[perm]
        out["U" + tag] = u.astype(ml_dtypes.bfloat16)
    out["LW"] = np.ascontiguousarray(
        np.asarray(weights["lin_W"])[0].reshape(KC, P).T
    ).astype(ml_dtypes.bfloat16)
    out["LB"] = np.asarray(weights["lin_b"]).reshape(1, 1).astype(np.float32)
    return out


_CACHE = {}
_LAST_RESULTS = None


def kernel(**inputs) -> np.ndarray:
    global _LAST_RESULTS
    key = "full"
    if key not in _CACHE:
        _CACHE[key] = build_nc(T, T)
    nc = _CACHE[key]

    x = np.asarray(inputs["x"], dtype=np.float32)
    in_maps = [
        prep_core_inputs(x[i * BL : (i + 1) * BL], inputs) for i in range(N_CORES)
    ]

    res = run_bass_kernel_spmd(nc, in_maps, core_ids=list(range(N_CORES)))
    _LAST_RESULTS = res
    y = np.empty((B, T, 1), dtype=np.float32)
    for i in range(N_CORES):
        yi = np.asarray(res.results[i]["Y"], dtype=np.float32).reshape(T, BL)
        y[i * BL : (i + 1) * BL, :, 0] = yi.T
    return y
